# revision 10
# baseline (speedup 1.0000x reference)
"""Trainium2 Bass kernel for CrossDomainAspectLabelPropagation (moe_routing).

Strategy (expert-parallel dispatch):
  - Host sorts the batch by domain_id; core c gets exactly the rows of
    domain c, padded to P = max domain count (rounded up to 16).
  - Each core runs the whole network on its rows with ONLY its own
    expert's dom-encoder weights -> dom costs 1x batch instead of Dx.
  - Activations live in SBUF in TRANSPOSED layout [feature, row]: weights
    are the stationary matmul operand, activations stream, so chained
    layers need no transposes.  All matmuls are float32r (full PE rate at
    free-dim >= 256, ~1e-4 relative error).
  - LayerNorm across features (= partition dim x 8 chunks) is computed
    with ones-matmuls on the PE: mu/E[y^2] accumulate in PSUM broadcast
    to all 128 partitions; apply is 2 DVE ops per chunk.  The LN affine
    (g, be) is folded into downstream weights host-side; the returned
    inv/dom get the affine applied on the host.
  - Outputs are gathered, un-padded, inverse-permuted on the host.
"""
import os
import sys

for _p in ("/opt/trn_rl_repo", "/root/.axon_site/_ro/trn_rl_repo"):
    if os.path.isdir(_p) and _p not in sys.path:
        sys.path.append(_p)

import numpy as np

H = 1024
D = 8
NCORES = 8
LN_EPS = 1e-5

# Filled with the profiled exec time (ns) when KERNEL_TRACE=1.
LAST_EXEC_TIME_NS = None

_build_cache = {}


def _col_blocks(P):
    """Split P columns into blocks each <=512 and >=256 (fp32r full-rate)."""
    assert P >= 256, P
    blocks = []
    rem = P
    while rem > 512:
        if rem - 512 >= 256 or rem == 1024:
            blocks.append(512)
            rem -= 512
        else:
            # rem in (512, 768): split into two blocks >= 256
            a = (rem // 2 + 7) & ~7
            blocks.append(a)
            rem -= a
    blocks.append(rem)
    assert sum(blocks) == P and all(256 <= b <= 512 for b in blocks), blocks
    return blocks


def _build(P):
    import concourse.bacc as bacc
    import concourse.tile as tile
    from concourse import mybir

    F32 = mybir.dt.float32
    F32R = mybir.dt.float32r
    AF = mybir.ActivationFunctionType

    blocks = _col_blocks(P)
    boff = [0]
    for b in blocks:
        boff.append(boff[-1] + b)

    nc = bacc.Bacc("TRN2", target_bir_lowering=False, debug=False,
                   num_devices=NCORES)

    # ---- DRAM I/O ----------------------------------------------------
    xT = nc.declare_dram_parameter("xT", [H, P], F32R, isOutput=False)
    w_names = ["w1i", "w2i", "w1v", "w2v", "w1d", "w2d"]
    w_dram = {n: nc.declare_dram_parameter(n, [H, H], F32R, isOutput=False)
              for n in w_names}
    wc1 = nc.declare_dram_parameter("wc1", [2 * H, H], F32R, isOutput=False)
    wc2p = nc.declare_dram_parameter("wc2p", [128, 8 * 3], F32R, isOutput=False)
    wd1 = nc.declare_dram_parameter("wd1", [H, H // 2], F32R, isOutput=False)
    wd2p = nc.declare_dram_parameter("wd2p", [128, 4 * 8], F32R, isOutput=False)
    # packed per-feature vectors: [128, 8] with column j = features j*128..
    b1i = nc.declare_dram_parameter("b1i", [128, 8], F32, isOutput=False)
    b2i = nc.declare_dram_parameter("b2i", [128, 8], F32, isOutput=False)
    b1v = nc.declare_dram_parameter("b1v", [128, 8], F32, isOutput=False)
    b2v = nc.declare_dram_parameter("b2v", [128, 8], F32, isOutput=False)
    b1d = nc.declare_dram_parameter("b1d", [128, 8], F32, isOutput=False)
    b2d = nc.declare_dram_parameter("b2d", [128, 8], F32, isOutput=False)
    bc1 = nc.declare_dram_parameter("bc1", [128, 8], F32, isOutput=False)
    bc2 = nc.declare_dram_parameter("bc2", [128, 1], F32, isOutput=False)
    bd1 = nc.declare_dram_parameter("bd1", [128, 4], F32, isOutput=False)
    bd2 = nc.declare_dram_parameter("bd2", [128, 1], F32, isOutput=False)
    ones_in = nc.declare_dram_parameter("ones_in", [128, 128], F32R,
                                        isOutput=False)

    invT_o = nc.declare_dram_parameter("invT", [H, P], F32R, isOutput=True)
    domT_o = nc.declare_dram_parameter("domT", [H, P], F32R, isOutput=True)
    aspT_o = nc.declare_dram_parameter("aspT", [3, P], F32, isOutput=True)
    dpT_o = nc.declare_dram_parameter("dpT", [8, P], F32, isOutput=True)

    with tile.TileContext(nc) as tc:
        with (
            tc.tile_pool(name="acts", bufs=1) as acts,
            tc.tile_pool(name="consts", bufs=1) as consts,
            tc.tile_pool(name="wp", bufs=10) as wp,
            tc.tile_pool(name="psp", bufs=8, space="PSUM") as psp,
            tc.tile_pool(name="stp", bufs=6) as stp,
            tc.tile_pool(name="sqp", bufs=9) as sqp,
            tc.tile_pool(name="hdp", bufs=5) as hdp,
            tc.tile_pool(name="outp", bufs=1) as outp,
        ):
            # persistent activation buffers [128, 8, P]
            A = acts.tile([128, 8, P], F32R, tag="A")   # x -> var -> dom
            B = acts.tile([128, 8, P], F32R, tag="B")   # h_* / h_cls
            C = acts.tile([128, 8, P], F32R, tag="C")   # inv

            # constants
            ones_sb = consts.tile([128, 128], F32R, tag="ones")
            nc.sync.dma_start(out=ones_sb[:], in_=ones_in[:])
            bias_sb = {}
            for name, t in (("b1i", b1i), ("b2i", b2i), ("b1v", b1v),
                            ("b2v", b2v), ("b1d", b1d), ("b2d", b2d),
                            ("bc1", bc1)):
                bt = consts.tile([128, 8], F32, tag=name)
                nc.sync.dma_start(out=bt[:], in_=t[:])
                bias_sb[name] = bt
            bd1_sb = consts.tile([128, 4], F32, tag="bd1")
            nc.sync.dma_start(out=bd1_sb[:], in_=bd1[:])
            bc2_sb = consts.tile([128, 1], F32, tag="bc2")
            nc.sync.dma_start(out=bc2_sb[:], in_=bc2[:])
            bd2_sb = consts.tile([128, 1], F32, tag="bd2")
            nc.sync.dma_start(out=bd2_sb[:], in_=bd2[:])
            wc2_sb = consts.tile([128, 24], F32R, tag="wc2")
            nc.sync.dma_start(out=wc2_sb[:], in_=wc2p[:])
            wd2_sb = consts.tile([128, 32], F32R, tag="wd2")
            nc.sync.dma_start(out=wd2_sb[:], in_=wd2p[:])
            eps_sb = consts.tile([128, 1], F32, tag="eps")
            nc.vector.memset(eps_sb[:], float(LN_EPS))
            # PE warm-up: keeps the PE busy (HAM at full clock) while the
            # first-layer weights + activations stream in from HBM.
            warm_ps = psp.tile([128, 128], F32, tag="ps")
            for _ in range(56):
                nc.tensor.matmul(warm_ps[:], lhsT=ones_sb[:],
                                 rhs=ones_sb[:], start=True, stop=True)

            def load_w(dram, nrows=H, width=H):
                tiles = []
                for h in range(nrows // 128):
                    wt = wp.tile([128, width], F32R, tag="w")
                    nc.sync.dma_start(
                        out=wt[:], in_=dram[h * 128:(h + 1) * 128, :])
                    tiles.append(wt)
                return tiles

            def relu_layer(dst, src, wname, bias_tile, wt=None):
                """dst[:, f, :] = relu(W^T src + b) ; W [H, H] from dram."""
                if wt is None:
                    wt = load_w(w_dram[wname])
                for bi, bn in enumerate(blocks):
                    lo, hi = boff[bi], boff[bi + 1]
                    for f in range(8):
                        ps = psp.tile([128, bn], F32, tag="ps")
                        for h in range(8):
                            nc.tensor.matmul(
                                ps[:],
                                lhsT=wt[h][:, f * 128:(f + 1) * 128],
                                rhs=src[:, h, lo:hi],
                                start=(h == 0), stop=(h == 7))
                        nc.scalar.activation(
                            dst[:, f, lo:hi], ps[:], AF.Relu,
                            bias=bias_tile[:, f:f + 1])

            def ln_layer(dst, src, wname, bias_tile):
                """dst = LN_core(W^T src + b2): (y - mu) * rsqrt(var+eps).

                (per-feature g/be affine is folded into downstream weights
                host-side.)"""
                wt = load_w(w_dram[wname])
                for bi, bn in enumerate(blocks):
                    lo, hi = boff[bi], boff[bi + 1]
                    sqs = []
                    for f in range(8):
                        ps = psp.tile([128, bn], F32, tag="ps")
                        for h in range(8):
                            nc.tensor.matmul(
                                ps[:],
                                lhsT=wt[h][:, f * 128:(f + 1) * 128],
                                rhs=src[:, h, lo:hi],
                                start=(h == 0), stop=(h == 7))
                        # y (with b2) into dst as scratch
                        nc.scalar.activation(
                            dst[:, f, lo:hi], ps[:], AF.Identity,
                            bias=bias_tile[:, f:f + 1])
                        sq = sqp.tile([128, bn], F32R, tag="sq",
                                      name=f"sq{bi}_{f}")
                        nc.vector.tensor_mul(sq[:], dst[:, f, lo:hi],
                                             dst[:, f, lo:hi])
                        sqs.append(sq)
                    # stats after all mains: PE only waits on the last chunk
                    mu_ps = psp.tile([128, bn], F32, tag="ps")
                    sq_ps = psp.tile([128, bn], F32, tag="ps")
                    for f in range(8):
                        nc.tensor.matmul(mu_ps[:], lhsT=ones_sb[:],
                                         rhs=dst[:, f, lo:hi],
                                         start=(f == 0), stop=(f == 7))
                        nc.tensor.matmul(sq_ps[:], lhsT=ones_sb[:],
                                         rhs=sqs[f][:],
                                         start=(f == 0), stop=(f == 7))
                    # var = E[y^2] - mu^2 ; rho = 1/sqrt(var+eps)
                    mu2 = stp.tile([128, bn], F32, tag="st")
                    nc.scalar.activation(mu2[:], mu_ps[:], AF.Square)
                    var = stp.tile([128, bn], F32, tag="st")
                    nc.vector.tensor_sub(var[:], sq_ps[:], mu2[:])
                    sd = stp.tile([128, bn], F32, tag="st")
                    nc.scalar.activation(sd[:], var[:], AF.Sqrt,
                                         bias=eps_sb[:, 0:1])
                    rho = stp.tile([128, bn], F32, tag="st")
                    scr = stp.tile([128, bn], F32, tag="st")
                    nc.vector.reciprocal_approx_accurate(rho[:], sd[:],
                                                         scratch=scr[:])
                    for f in range(8):
                        nc.vector.tensor_sub(dst[:, f, lo:hi],
                                             dst[:, f, lo:hi], mu_ps[:])
                        nc.vector.tensor_mul(dst[:, f, lo:hi],
                                             dst[:, f, lo:hi], rho[:])

            # ---- encoders -------------------------------------------
            # first-layer weights interleaved with x block 0 so L1's first
            # block only waits for ~6MB of critical DMA
            wt1 = []
            for h in range(8):
                w0 = wp.tile([128, H], F32R, tag="w", name=f"w1i{h}")
                nc.sync.dma_start(
                    out=w0[:], in_=w_dram["w1i"][h * 128:(h + 1) * 128, :])
                wt1.append(w0)
                nc.sync.dma_start(out=A[:, h, boff[0]:boff[1]],
                                  in_=xT[h * 128:(h + 1) * 128,
                                         boff[0]:boff[1]])
            for bi in range(1, len(blocks)):
                lo, hi = boff[bi], boff[bi + 1]
                for h in range(8):
                    nc.sync.dma_start(out=A[:, h, lo:hi],
                                      in_=xT[h * 128:(h + 1) * 128, lo:hi])
            relu_layer(B, A, "w1i", bias_sb["b1i"], wt=wt1)
            ln_layer(C, B, "w2i", bias_sb["b2i"])           # C = inv_core
            relu_layer(B, A, "w1v", bias_sb["b1v"])         # (w1v g_i-folded? no: x raw)
            ln_layer(A, B, "w2v", bias_sb["b2v"])           # A = var_core
            relu_layer(B, A, "w1d", bias_sb["b1d"])         # w1d folded with g_v
            ln_layer(A, B, "w2d", bias_sb["b2d"])           # A = dom_core

            # PE filler: keeps PE busy/warm while wc1 block-0 streams in
            fill_ps = psp.tile([128, 128], F32, tag="ps")
            for _ in range(40):
                nc.tensor.matmul(fill_ps[:], lhsT=ones_sb[:],
                                 rhs=ones_sb[:], start=True, stop=True)

            # ---- classifier -----------------------------------------
            # h_cls = relu(Wc1a^T inv + Wc1b^T dom + bc1) -> B
            asp_sb = outp.tile([3, P], F32, tag="asp")
            for bi, bn in enumerate(blocks):
                lo, hi = boff[bi], boff[bi + 1]
                cls_ps = [psp.tile([128, bn], F32, tag="ps", name=f"clsps{bi}_{i}") for i in range(8)]
                for k in range(16):
                    wt = wp.tile([128, H], F32R, tag="w")
                    nc.sync.dma_start(
                        out=wt[:], in_=wc1[k * 128:(k + 1) * 128, :])
                    src = C if k < 8 else A
                    for f in range(8):
                        nc.tensor.matmul(
                            cls_ps[f][:],
                            lhsT=wt[:, f * 128:(f + 1) * 128],
                            rhs=src[:, k % 8, lo:hi],
                            start=(k == 0), stop=(k == 15))
                for f in range(8):
                    nc.scalar.activation(
                        B[:, f, lo:hi], cls_ps[f][:], AF.Relu,
                        bias=bias_sb["bc1"][:, f:f + 1])
                # aspect = Wc2^T h_cls + bc2
                asp_ps = psp.tile([3, bn], F32, tag="ps")
                for f in range(8):
                    nc.tensor.matmul(
                        asp_ps[:], lhsT=wc2_sb[:, 3 * f:3 * f + 3],
                        rhs=B[:, f, lo:hi],
                        start=(f == 0), stop=(f == 7))
                nc.scalar.activation(asp_sb[:, lo:hi], asp_ps[:], AF.Identity,
                                     bias=bc2_sb[:3, :])
            nc.gpsimd.dma_start(out=aspT_o[:], in_=asp_sb[:])

            # write inv/dom (core LN values; host applies g/be) — after the
            # classifier so these stores don't compete with wc1 loads
            for h in range(8):
                nc.gpsimd.dma_start(out=invT_o[h * 128:(h + 1) * 128, :],
                                    in_=C[:, h, :])
                nc.gpsimd.dma_start(out=domT_o[h * 128:(h + 1) * 128, :],
                                    in_=A[:, h, :])

            # PE filler: covers the wd1 stream ramp after cls
            fill2_ps = psp.tile([128, 128], F32, tag="ps")
            for _ in range(24):
                nc.tensor.matmul(fill2_ps[:], lhsT=ones_sb[:],
                                 rhs=ones_sb[:], start=True, stop=True)

            # ---- discriminator --------------------------------------
            dp_sb = outp.tile([8, P], F32, tag="dp")
            wdt = load_w(wd1, nrows=H, width=H // 2)
            for bi, bn in enumerate(blocks):
                lo, hi = boff[bi], boff[bi + 1]
                dp_ps = psp.tile([8, bn], F32, tag="ps")
                hds = []
                for f in range(4):
                    ps = psp.tile([128, bn], F32, tag="ps")
                    for h in range(8):
                        nc.tensor.matmul(
                            ps[:], lhsT=wdt[h][:, f * 128:(f + 1) * 128],
                            rhs=C[:, h, lo:hi],
                            start=(h == 0), stop=(h == 7))
                    hd = hdp.tile([128, bn], F32R, tag="hd",
                                  name=f"hd{bi}_{f}")
                    nc.scalar.activation(hd[:], ps[:], AF.Relu,
                                         bias=bd1_sb[:, f:f + 1])
                    hds.append(hd)
                for f in range(4):
                    nc.tensor.matmul(
                        dp_ps[:], lhsT=wd2_sb[:, 8 * f:8 * f + 8],
                        rhs=hds[f][:], start=(f == 0), stop=(f == 3))
                nc.scalar.activation(dp_sb[:, lo:hi], dp_ps[:], AF.Identity,
                                     bias=bd2_sb[:8, :])
            nc.gpsimd.dma_start(out=dpT_o[:], in_=dp_sb[:])

    nc.compile()
    return nc


def _pack_vec(v, cols):
    """[cols*128] -> [128, cols] with column j = v[j*128:(j+1)*128]."""
    return np.ascontiguousarray(np.asarray(v, np.float32).reshape(cols, 128).T)


def kernel(features, domain_ids, params):
    global LAST_EXEC_TIME_NS
    from concourse.bass_utils import run_bass_kernel_spmd

    features = np.asarray(features, np.float32)
    domain_ids = np.asarray(domain_ids).astype(np.int64)
    B = features.shape[0]

    def g(*ks):
        p = params
        for k in ks:
            p = p[k]
        return np.asarray(p, np.float32)

    # ---- host routing ----------------------------------------------
    order = np.argsort(domain_ids, kind="stable")
    sorted_ids = domain_ids[order]
    counts = np.bincount(domain_ids, minlength=D).astype(np.int64)
    offs = np.concatenate([[0], np.cumsum(counts)])
    P = max(int(-(-counts.max() // 16) * 16), 256)

    nc = _build_cache.get(P)
    if nc is None:
        nc = _build(P)
        _build_cache[P] = nc

    gi, bei = g("inv", "g"), g("inv", "be")
    gv, bev = g("var", "g"), g("var", "be")
    gd_all, bed_all = g("dom", "g"), g("dom", "be")  # [D, H]
    w1v_r = g("var", "W1")
    wd1_r = g("disc", "W1")
    wc1_r = g("cls", "W1")
    wc2_r = g("cls", "W2")

    in_maps = []
    for c in range(D):
        rows = order[offs[c]:offs[c + 1]]
        x = np.zeros((P, H), np.float32)
        x[: len(rows)] = features[rows]
        gd, bed = gd_all[c], bed_all[c]
        # fold LN affines into consumers:
        #   w1d consumes var: w1d' = g_v[:,None] * w1d ; b1d' += w1d^T be_v
        w1d_f = g("dom", "W1")[c] * gv[:, None]
        b1d_f = g("dom", "b1")[c] + g("dom", "W1")[c].T @ bev
        #   wc1 top consumes inv, bottom consumes dom
        wc1_f = np.concatenate([wc1_r[:H] * gi[:, None],
                                wc1_r[H:] * gd[:, None]], axis=0)
        bc1_f = (g("cls", "b1") + wc1_r[:H].T @ bei + wc1_r[H:].T @ bed)
        #   wd1 consumes inv
        wd1_f = wd1_r * gi[:, None]
        bd1_f = g("disc", "b1") + wd1_r.T @ bei

        m = {
            "xT": np.ascontiguousarray(x.T),
            "w1i": g("inv", "W1"), "w2i": g("inv", "W2"),
            "w1v": w1v_r, "w2v": g("var", "W2"),
            "w1d": np.ascontiguousarray(w1d_f),
            "w2d": g("dom", "W2")[c],
            "wc1": np.ascontiguousarray(wc1_f),
            "wc2p": np.ascontiguousarray(
                wc2_r.reshape(8, 128, 3).transpose(1, 0, 2).reshape(128, 24)),
            "wd1": np.ascontiguousarray(wd1_f),
            "wd2p": np.ascontiguousarray(
                g("disc", "W2").reshape(4, 128, 8).transpose(1, 0, 2)
                .reshape(128, 32)),
            "b1i": _pack_vec(g("inv", "b1"), 8),
            "b2i": _pack_vec(g("inv", "b2"), 8),
            "b1v": _pack_vec(g("var", "b1"), 8),
            "b2v": _pack_vec(g("var", "b2"), 8),
            "b1d": _pack_vec(b1d_f, 8),
            "b2d": _pack_vec(g("dom", "b2")[c], 8),
            "bc1": _pack_vec(bc1_f, 8),
            "bc2": np.ascontiguousarray(
                np.pad(g("cls", "b2"), (0, 125)).reshape(128, 1)),
            "bd1": _pack_vec(bd1_f, 4),
            "bd2": np.ascontiguousarray(
                np.pad(g("disc", "b2"), (0, 120)).reshape(128, 1)),
            "ones_in": np.full((128, 128), 1.0 / H, np.float32),
        }
        in_maps.append(m)

    trace = os.environ.get("KERNEL_TRACE", "0") == "1"
    res = run_bass_kernel_spmd(nc, in_maps, core_ids=list(range(NCORES)),
                               trace=trace)
    if trace:
        LAST_EXEC_TIME_NS = res.exec_time_ns

    # ---- host gather / epilogue ------------------------------------
    aspect = np.empty((B, 3), np.float32)
    dpred = np.empty((B, D), np.float32)
    inv = np.empty((B, H), np.float32)
    dom = np.empty((B, H), np.float32)
    for c in range(D):
        rows = order[offs[c]:offs[c + 1]]
        n = len(rows)
        r = res.results[c]
        aspect[rows] = r["aspT"].T[:n]
        dpred[rows] = r["dpT"].T[:n]
        inv[rows] = r["invT"].T[:n] * gi[None, :] + bei[None, :]
        dom[rows] = r["domT"].T[:n] * gd_all[c][None, :] + bed_all[c][None, :]
    return aspect, dpred, inv, dom


# revision 11
# speedup vs baseline: 1.0299x; 1.0299x over previous
"""Trainium2 Bass kernel for CrossDomainAspectLabelPropagation (moe_routing).

Strategy (expert-parallel dispatch):
  - Host sorts the batch by domain_id; core c gets exactly the rows of
    domain c, padded to P = max domain count (rounded up to 16).
  - Each core runs the whole network on its rows with ONLY its own
    expert's dom-encoder weights -> dom costs 1x batch instead of Dx.
  - Activations live in SBUF in TRANSPOSED layout [feature, row]: weights
    are the stationary matmul operand, activations stream, so chained
    layers need no transposes.  All matmuls are float32r (full PE rate at
    free-dim >= 256, ~1e-4 relative error).
  - LayerNorm across features (= partition dim x 8 chunks) is computed
    with ones-matmuls on the PE: mu/E[y^2] accumulate in PSUM broadcast
    to all 128 partitions; apply is 2 DVE ops per chunk.  The LN affine
    (g, be) is folded into downstream weights host-side; the returned
    inv/dom get the affine applied on the host.
  - Outputs are gathered, un-padded, inverse-permuted on the host.
"""
import os
import sys

for _p in ("/opt/trn_rl_repo", "/root/.axon_site/_ro/trn_rl_repo"):
    if os.path.isdir(_p) and _p not in sys.path:
        sys.path.append(_p)

import numpy as np

H = 1024
D = 8
NCORES = 8
LN_EPS = 1e-5

# Filled with the profiled exec time (ns) when KERNEL_TRACE=1.
LAST_EXEC_TIME_NS = None

_build_cache = {}


def _col_blocks(P):
    """Split P columns into blocks each <=512 and >=256 (fp32r full-rate)."""
    assert P >= 256, P
    blocks = []
    rem = P
    while rem > 512:
        if rem - 512 >= 256 or rem == 1024:
            blocks.append(512)
            rem -= 512
        else:
            # rem in (512, 768): split into two blocks >= 256
            a = (rem // 2 + 7) & ~7
            blocks.append(a)
            rem -= a
    blocks.append(rem)
    assert sum(blocks) == P and all(256 <= b <= 512 for b in blocks), blocks
    return blocks


def _build(P):
    import concourse.bacc as bacc
    import concourse.tile as tile
    from concourse import mybir

    F32 = mybir.dt.float32
    F32R = mybir.dt.float32r
    AF = mybir.ActivationFunctionType

    blocks = _col_blocks(P)
    boff = [0]
    for b in blocks:
        boff.append(boff[-1] + b)

    nc = bacc.Bacc("TRN2", target_bir_lowering=False, debug=False,
                   num_devices=NCORES)

    # ---- DRAM I/O ----------------------------------------------------
    xT = nc.declare_dram_parameter("xT", [H, P], F32R, isOutput=False)
    w_names = ["w1i", "w2i", "w1v", "w2v", "w1d", "w2d"]
    w_dram = {n: nc.declare_dram_parameter(n, [H, H], F32R, isOutput=False)
              for n in w_names}
    wc1 = nc.declare_dram_parameter("wc1", [2 * H, H], F32R, isOutput=False)
    wc2p = nc.declare_dram_parameter("wc2p", [128, 8 * 3], F32R, isOutput=False)
    wd1 = nc.declare_dram_parameter("wd1", [H, H // 2], F32R, isOutput=False)
    wd2p = nc.declare_dram_parameter("wd2p", [128, 4 * 8], F32R, isOutput=False)
    # packed per-feature vectors: [128, 8] with column j = features j*128..
    b1i = nc.declare_dram_parameter("b1i", [128, 8], F32, isOutput=False)
    b2i = nc.declare_dram_parameter("b2i", [128, 8], F32, isOutput=False)
    b1v = nc.declare_dram_parameter("b1v", [128, 8], F32, isOutput=False)
    b2v = nc.declare_dram_parameter("b2v", [128, 8], F32, isOutput=False)
    b1d = nc.declare_dram_parameter("b1d", [128, 8], F32, isOutput=False)
    b2d = nc.declare_dram_parameter("b2d", [128, 8], F32, isOutput=False)
    bc1 = nc.declare_dram_parameter("bc1", [128, 8], F32, isOutput=False)
    bc2 = nc.declare_dram_parameter("bc2", [128, 1], F32, isOutput=False)
    bd1 = nc.declare_dram_parameter("bd1", [128, 4], F32, isOutput=False)
    bd2 = nc.declare_dram_parameter("bd2", [128, 1], F32, isOutput=False)
    ones_in = nc.declare_dram_parameter("ones_in", [128, 128], F32R,
                                        isOutput=False)

    invT_o = nc.declare_dram_parameter("invT", [H, P], F32R, isOutput=True)
    domT_o = nc.declare_dram_parameter("domT", [H, P], F32R, isOutput=True)
    aspT_o = nc.declare_dram_parameter("aspT", [3, P], F32, isOutput=True)
    dpT_o = nc.declare_dram_parameter("dpT", [8, P], F32, isOutput=True)

    with tile.TileContext(nc) as tc:
        with (
            tc.tile_pool(name="acts", bufs=1) as acts,
            tc.tile_pool(name="consts", bufs=1) as consts,
            tc.tile_pool(name="wp", bufs=10) as wp,
            tc.tile_pool(name="psp", bufs=8, space="PSUM") as psp,
            tc.tile_pool(name="stp", bufs=6) as stp,
            tc.tile_pool(name="sqp", bufs=9) as sqp,
            tc.tile_pool(name="hdp", bufs=5) as hdp,
            tc.tile_pool(name="outp", bufs=1) as outp,
        ):
            # persistent activation buffers [128, 8, P]
            A = acts.tile([128, 8, P], F32R, tag="A")   # x -> var -> dom
            B = acts.tile([128, 8, P], F32R, tag="B")   # h_* / h_cls
            C = acts.tile([128, 8, P], F32R, tag="C")   # inv

            # constants
            ones_sb = consts.tile([128, 128], F32R, tag="ones")
            nc.sync.dma_start(out=ones_sb[:], in_=ones_in[:])
            bias_sb = {}
            for name, t in (("b1i", b1i), ("b2i", b2i), ("b1v", b1v),
                            ("b2v", b2v), ("b1d", b1d), ("b2d", b2d),
                            ("bc1", bc1)):
                bt = consts.tile([128, 8], F32, tag=name)
                nc.sync.dma_start(out=bt[:], in_=t[:])
                bias_sb[name] = bt
            bd1_sb = consts.tile([128, 4], F32, tag="bd1")
            nc.sync.dma_start(out=bd1_sb[:], in_=bd1[:])
            bc2_sb = consts.tile([128, 1], F32, tag="bc2")
            nc.sync.dma_start(out=bc2_sb[:], in_=bc2[:])
            bd2_sb = consts.tile([128, 1], F32, tag="bd2")
            nc.sync.dma_start(out=bd2_sb[:], in_=bd2[:])
            wc2_sb = consts.tile([128, 24], F32R, tag="wc2")
            nc.sync.dma_start(out=wc2_sb[:], in_=wc2p[:])
            wd2_sb = consts.tile([128, 32], F32R, tag="wd2")
            nc.sync.dma_start(out=wd2_sb[:], in_=wd2p[:])
            eps_sb = consts.tile([128, 1], F32, tag="eps")
            nc.vector.memset(eps_sb[:], float(LN_EPS))
            # PE warm-up: keeps the PE busy (HAM at full clock) while the
            # first-layer weights + activations stream in from HBM.
            warm_ps = psp.tile([128, 128], F32, tag="ps")
            for _ in range(56):
                nc.tensor.matmul(warm_ps[:], lhsT=ones_sb[:],
                                 rhs=ones_sb[:], start=True, stop=True)

            def load_w(dram, nrows=H, width=H):
                tiles = []
                for h in range(nrows // 128):
                    wt = wp.tile([128, width], F32R, tag="w")
                    nc.sync.dma_start(
                        out=wt[:], in_=dram[h * 128:(h + 1) * 128, :])
                    tiles.append(wt)
                return tiles

            def relu_layer(dst, src, wname, bias_tile, wt=None):
                """dst[:, f, :] = relu(W^T src + b) ; W [H, H] from dram."""
                if wt is None:
                    wt = load_w(w_dram[wname])
                for bi, bn in enumerate(blocks):
                    lo, hi = boff[bi], boff[bi + 1]
                    for f in range(8):
                        ps = psp.tile([128, bn], F32, tag="ps")
                        for h in range(8):
                            nc.tensor.matmul(
                                ps[:],
                                lhsT=wt[h][:, f * 128:(f + 1) * 128],
                                rhs=src[:, h, lo:hi],
                                start=(h == 0), stop=(h == 7))
                        nc.scalar.activation(
                            dst[:, f, lo:hi], ps[:], AF.Relu,
                            bias=bias_tile[:, f:f + 1])

            def ln_layer(dst, src, wname, bias_tile):
                """dst = LN_core(W^T src + b2): (y - mu) * rsqrt(var+eps).

                (per-feature g/be affine is folded into downstream weights
                host-side.)"""
                wt = load_w(w_dram[wname])
                for bi, bn in enumerate(blocks):
                    lo, hi = boff[bi], boff[bi + 1]
                    sqs = []
                    for f in range(8):
                        ps = psp.tile([128, bn], F32, tag="ps")
                        for h in range(8):
                            nc.tensor.matmul(
                                ps[:],
                                lhsT=wt[h][:, f * 128:(f + 1) * 128],
                                rhs=src[:, h, lo:hi],
                                start=(h == 0), stop=(h == 7))
                        # y (with b2) into dst as scratch
                        nc.scalar.activation(
                            dst[:, f, lo:hi], ps[:], AF.Identity,
                            bias=bias_tile[:, f:f + 1])
                        sq = sqp.tile([128, bn], F32R, tag="sq",
                                      name=f"sq{bi}_{f}")
                        nc.vector.tensor_mul(sq[:], dst[:, f, lo:hi],
                                             dst[:, f, lo:hi])
                        sqs.append(sq)
                    # stats after all mains: PE only waits on the last chunk
                    mu_ps = psp.tile([128, bn], F32, tag="ps")
                    sq_ps = psp.tile([128, bn], F32, tag="ps")
                    for f in range(8):
                        nc.tensor.matmul(mu_ps[:], lhsT=ones_sb[:],
                                         rhs=dst[:, f, lo:hi],
                                         start=(f == 0), stop=(f == 7))
                        nc.tensor.matmul(sq_ps[:], lhsT=ones_sb[:],
                                         rhs=sqs[f][:],
                                         start=(f == 0), stop=(f == 7))
                    # var = E[y^2] - mu^2 ; rho = 1/sqrt(var+eps)
                    mu2 = stp.tile([128, bn], F32, tag="st")
                    nc.scalar.activation(mu2[:], mu_ps[:], AF.Square)
                    var = stp.tile([128, bn], F32, tag="st")
                    nc.vector.tensor_sub(var[:], sq_ps[:], mu2[:])
                    sd = stp.tile([128, bn], F32, tag="st")
                    nc.scalar.activation(sd[:], var[:], AF.Sqrt,
                                         bias=eps_sb[:, 0:1])
                    rho = stp.tile([128, bn], F32, tag="st")
                    scr = stp.tile([128, bn], F32, tag="st")
                    nc.vector.reciprocal_approx_accurate(rho[:], sd[:],
                                                         scratch=scr[:])
                    for f in range(8):
                        nc.vector.tensor_sub(dst[:, f, lo:hi],
                                             dst[:, f, lo:hi], mu_ps[:])
                        nc.vector.tensor_mul(dst[:, f, lo:hi],
                                             dst[:, f, lo:hi], rho[:])

            # ---- encoders -------------------------------------------
            # first-layer weights interleaved with x block 0 so L1's first
            # block only waits for ~6MB of critical DMA
            wt1 = []
            for h in range(8):
                w0 = wp.tile([128, H], F32R, tag="w", name=f"w1i{h}")
                nc.sync.dma_start(
                    out=w0[:], in_=w_dram["w1i"][h * 128:(h + 1) * 128, :])
                wt1.append(w0)
                nc.sync.dma_start(out=A[:, h, boff[0]:boff[1]],
                                  in_=xT[h * 128:(h + 1) * 128,
                                         boff[0]:boff[1]])
            for bi in range(1, len(blocks)):
                lo, hi = boff[bi], boff[bi + 1]
                for h in range(8):
                    nc.sync.dma_start(out=A[:, h, lo:hi],
                                      in_=xT[h * 128:(h + 1) * 128, lo:hi])
            relu_layer(B, A, "w1i", bias_sb["b1i"], wt=wt1)
            ln_layer(C, B, "w2i", bias_sb["b2i"])           # C = inv_core
            relu_layer(B, A, "w1v", bias_sb["b1v"])         # (w1v g_i-folded? no: x raw)
            ln_layer(A, B, "w2v", bias_sb["b2v"])           # A = var_core
            relu_layer(B, A, "w1d", bias_sb["b1d"])         # w1d folded with g_v
            ln_layer(A, B, "w2d", bias_sb["b2d"])           # A = dom_core

            # ---- classifier -----------------------------------------
            # h_cls = relu(Wc1a^T inv + Wc1b^T dom + bc1) -> B
            asp_sb = outp.tile([3, P], F32, tag="asp")
            for bi, bn in enumerate(blocks):
                lo, hi = boff[bi], boff[bi + 1]
                cls_ps = [psp.tile([128, bn], F32, tag="ps", name=f"clsps{bi}_{i}") for i in range(8)]
                for k in range(16):
                    wt = wp.tile([128, H], F32R, tag="w")
                    nc.sync.dma_start(
                        out=wt[:], in_=wc1[k * 128:(k + 1) * 128, :])
                    src = C if k < 8 else A
                    for f in range(8):
                        nc.tensor.matmul(
                            cls_ps[f][:],
                            lhsT=wt[:, f * 128:(f + 1) * 128],
                            rhs=src[:, k % 8, lo:hi],
                            start=(k == 0), stop=(k == 15))
                for f in range(8):
                    nc.scalar.activation(
                        B[:, f, lo:hi], cls_ps[f][:], AF.Relu,
                        bias=bias_sb["bc1"][:, f:f + 1])
                # aspect = Wc2^T h_cls + bc2
                asp_ps = psp.tile([3, bn], F32, tag="ps")
                for f in range(8):
                    nc.tensor.matmul(
                        asp_ps[:], lhsT=wc2_sb[:, 3 * f:3 * f + 3],
                        rhs=B[:, f, lo:hi],
                        start=(f == 0), stop=(f == 7))
                nc.scalar.activation(asp_sb[:, lo:hi], asp_ps[:], AF.Identity,
                                     bias=bc2_sb[:3, :])
            nc.gpsimd.dma_start(out=aspT_o[:], in_=asp_sb[:])

            # write inv/dom (core LN values; host applies g/be) — after the
            # classifier so these stores don't compete with wc1 loads
            for h in range(8):
                nc.gpsimd.dma_start(out=invT_o[h * 128:(h + 1) * 128, :],
                                    in_=C[:, h, :])
                nc.gpsimd.dma_start(out=domT_o[h * 128:(h + 1) * 128, :],
                                    in_=A[:, h, :])

            # ---- discriminator --------------------------------------
            dp_sb = outp.tile([8, P], F32, tag="dp")
            wdt = load_w(wd1, nrows=H, width=H // 2)
            for bi, bn in enumerate(blocks):
                lo, hi = boff[bi], boff[bi + 1]
                dp_ps = psp.tile([8, bn], F32, tag="ps")
                hds = []
                for f in range(4):
                    ps = psp.tile([128, bn], F32, tag="ps")
                    for h in range(8):
                        nc.tensor.matmul(
                            ps[:], lhsT=wdt[h][:, f * 128:(f + 1) * 128],
                            rhs=C[:, h, lo:hi],
                            start=(h == 0), stop=(h == 7))
                    hd = hdp.tile([128, bn], F32R, tag="hd",
                                  name=f"hd{bi}_{f}")
                    nc.scalar.activation(hd[:], ps[:], AF.Relu,
                                         bias=bd1_sb[:, f:f + 1])
                    hds.append(hd)
                for f in range(4):
                    nc.tensor.matmul(
                        dp_ps[:], lhsT=wd2_sb[:, 8 * f:8 * f + 8],
                        rhs=hds[f][:], start=(f == 0), stop=(f == 3))
                nc.scalar.activation(dp_sb[:, lo:hi], dp_ps[:], AF.Identity,
                                     bias=bd2_sb[:8, :])
            nc.gpsimd.dma_start(out=dpT_o[:], in_=dp_sb[:])

    nc.compile()
    return nc


def _pack_vec(v, cols):
    """[cols*128] -> [128, cols] with column j = v[j*128:(j+1)*128]."""
    return np.ascontiguousarray(np.asarray(v, np.float32).reshape(cols, 128).T)


def kernel(features, domain_ids, params):
    global LAST_EXEC_TIME_NS
    from concourse.bass_utils import run_bass_kernel_spmd

    features = np.asarray(features, np.float32)
    domain_ids = np.asarray(domain_ids).astype(np.int64)
    B = features.shape[0]

    def g(*ks):
        p = params
        for k in ks:
            p = p[k]
        return np.asarray(p, np.float32)

    # ---- host routing ----------------------------------------------
    order = np.argsort(domain_ids, kind="stable")
    sorted_ids = domain_ids[order]
    counts = np.bincount(domain_ids, minlength=D).astype(np.int64)
    offs = np.concatenate([[0], np.cumsum(counts)])
    P = max(int(-(-counts.max() // 16) * 16), 256)

    nc = _build_cache.get(P)
    if nc is None:
        nc = _build(P)
        _build_cache[P] = nc

    gi, bei = g("inv", "g"), g("inv", "be")
    gv, bev = g("var", "g"), g("var", "be")
    gd_all, bed_all = g("dom", "g"), g("dom", "be")  # [D, H]
    w1v_r = g("var", "W1")
    wd1_r = g("disc", "W1")
    wc1_r = g("cls", "W1")
    wc2_r = g("cls", "W2")

    in_maps = []
    for c in range(D):
        rows = order[offs[c]:offs[c + 1]]
        x = np.zeros((P, H), np.float32)
        x[: len(rows)] = features[rows]
        gd, bed = gd_all[c], bed_all[c]
        # fold LN affines into consumers:
        #   w1d consumes var: w1d' = g_v[:,None] * w1d ; b1d' += w1d^T be_v
        w1d_f = g("dom", "W1")[c] * gv[:, None]
        b1d_f = g("dom", "b1")[c] + g("dom", "W1")[c].T @ bev
        #   wc1 top consumes inv, bottom consumes dom
        wc1_f = np.concatenate([wc1_r[:H] * gi[:, None],
                                wc1_r[H:] * gd[:, None]], axis=0)
        bc1_f = (g("cls", "b1") + wc1_r[:H].T @ bei + wc1_r[H:].T @ bed)
        #   wd1 consumes inv
        wd1_f = wd1_r * gi[:, None]
        bd1_f = g("disc", "b1") + wd1_r.T @ bei

        m = {
            "xT": np.ascontiguousarray(x.T),
            "w1i": g("inv", "W1"), "w2i": g("inv", "W2"),
            "w1v": w1v_r, "w2v": g("var", "W2"),
            "w1d": np.ascontiguousarray(w1d_f),
            "w2d": g("dom", "W2")[c],
            "wc1": np.ascontiguousarray(wc1_f),
            "wc2p": np.ascontiguousarray(
                wc2_r.reshape(8, 128, 3).transpose(1, 0, 2).reshape(128, 24)),
            "wd1": np.ascontiguousarray(wd1_f),
            "wd2p": np.ascontiguousarray(
                g("disc", "W2").reshape(4, 128, 8).transpose(1, 0, 2)
                .reshape(128, 32)),
            "b1i": _pack_vec(g("inv", "b1"), 8),
            "b2i": _pack_vec(g("inv", "b2"), 8),
            "b1v": _pack_vec(g("var", "b1"), 8),
            "b2v": _pack_vec(g("var", "b2"), 8),
            "b1d": _pack_vec(b1d_f, 8),
            "b2d": _pack_vec(g("dom", "b2")[c], 8),
            "bc1": _pack_vec(bc1_f, 8),
            "bc2": np.ascontiguousarray(
                np.pad(g("cls", "b2"), (0, 125)).reshape(128, 1)),
            "bd1": _pack_vec(bd1_f, 4),
            "bd2": np.ascontiguousarray(
                np.pad(g("disc", "b2"), (0, 120)).reshape(128, 1)),
            "ones_in": np.full((128, 128), 1.0 / H, np.float32),
        }
        in_maps.append(m)

    trace = os.environ.get("KERNEL_TRACE", "0") == "1"
    res = run_bass_kernel_spmd(nc, in_maps, core_ids=list(range(NCORES)),
                               trace=trace)
    if trace:
        LAST_EXEC_TIME_NS = res.exec_time_ns

    # ---- host gather / epilogue ------------------------------------
    aspect = np.empty((B, 3), np.float32)
    dpred = np.empty((B, D), np.float32)
    inv = np.empty((B, H), np.float32)
    dom = np.empty((B, H), np.float32)
    for c in range(D):
        rows = order[offs[c]:offs[c + 1]]
        n = len(rows)
        r = res.results[c]
        aspect[rows] = r["aspT"].T[:n]
        dpred[rows] = r["dpT"].T[:n]
        inv[rows] = r["invT"].T[:n] * gi[None, :] + bei[None, :]
        dom[rows] = r["domT"].T[:n] * gd_all[c][None, :] + bed_all[c][None, :]
    return aspect, dpred, inv, dom
